# revision 3
# baseline (speedup 1.0000x reference)
#!/usr/bin/env python3
"""MultiHeadCausalAttention on 8 trn2 NeuronCores.

Sharding: core c handles batch b = c//2 and head-group g = c%2 (8 of 16 heads,
Megatron-style column shard of Wq/Wk/Wv, row shard of Wo). The pair (2b, 2b+1)
all-reduces its partial output projection on-device.

Numerics: logit path (q, k projections and q.k^T scores) uses fp16 hi/lo
split operands with 3 accumulating matmuls per product (fp32-grade results at
full PE rate; measured ~3e-7 rel err). Value path (v, attn@v, Wo) uses single
fp16 (~3e-4 rel err). The softmax scale sqrt(2048) is folded into Wq on host.
Softmax: row max via DVE reduce (negated -> exp bias), exp on ACT with
accum_out row sums; normalization is folded into the p^T transpose by
multiplying with diag(1/sum) on the PE.
"""
import sys
sys.path.insert(0, "/opt/trn_rl_repo")

import numpy as np
from contextlib import ExitStack

import concourse.bass as bass
import concourse.bacc as bacc
import concourse.mybir as mybir
import concourse.tile as tile
from concourse.bass_utils import run_bass_kernel_spmd

F16 = mybir.dt.float16
F32 = mybir.dt.float32

B, T, DIN, DOUT = 4, 2048, 2048, 2048
H, DH = 16, 128
INV_SCALE = float(DOUT) ** 0.5
HPC = 8            # heads per core
SH = 4             # heads per subgroup
NSUB = HPC // SH   # 2 subgroups
TC = 512           # t/s chunk width
NCH = T // TC      # 4 chunks
KT = DIN // 128    # 16 contraction tiles
NSB = T // 128     # 16 s-blocks
GDIM = SH * DH     # 512: head-dim cols per subgroup

USE_COLLECTIVE = True

_cached = {}


def _strided3(dram_ap, row_stride, col_off, n_p, n_k, n_w, k_stride):
    """AP over a 2-D DRAM tensor: [partition(n_p rows), k(n_k row-tiles), w cols]."""
    return bass.AP(tensor=dram_ap.tensor, offset=dram_ap.offset + col_off,
                   ap=[[row_stride, n_p], [k_stride, n_k], [1, n_w]])


def build_nc(use_collective=USE_COLLECTIVE):
    nc = bacc.Bacc()

    dxh = nc.dram_tensor("xh", [DIN, T], F16, kind="ExternalInput")
    dxl = nc.dram_tensor("xl", [DIN, T], F16, kind="ExternalInput")
    dwqh = nc.dram_tensor("wqh", [DIN, HPC * DH], F16, kind="ExternalInput")
    dwql = nc.dram_tensor("wql", [DIN, HPC * DH], F16, kind="ExternalInput")
    dwkh = nc.dram_tensor("wkh", [DIN, HPC * DH], F16, kind="ExternalInput")
    dwkl = nc.dram_tensor("wkl", [DIN, HPC * DH], F16, kind="ExternalInput")
    dwv = nc.dram_tensor("wv", [DIN, HPC * DH], F16, kind="ExternalInput")
    dwo = nc.dram_tensor("wo", [HPC * DH, DOUT], F16, kind="ExternalInput")
    dmask = nc.dram_tensor("masks", [128, 896], F32, kind="ExternalInput")
    dident = nc.dram_tensor("ident", [128, 128], F16, kind="ExternalInput")

    dout = nc.dram_tensor("out", [T, DOUT], F32, kind="ExternalOutput")
    if use_collective:
        dpart = nc.dram_tensor("part", [T, DOUT], F32, kind="Internal")
        darout = nc.dram_tensor("arout", [T, DOUT], F32, kind="Internal")
    else:
        dpart = None
        darout = None

    with tile.TileContext(nc) as tc, ExitStack() as ctx:
        persist = ctx.enter_context(tc.tile_pool(name="persist", bufs=1))
        wpool = ctx.enter_context(tc.tile_pool(name="wpool", bufs=1))
        xpool = ctx.enter_context(tc.tile_pool(name="xpool", bufs=1))
        work = ctx.enter_context(tc.tile_pool(name="work", bufs=2))
        ptp = ctx.enter_context(tc.tile_pool(name="ptp", bufs=1))
        small = ctx.enter_context(tc.tile_pool(name="small", bufs=4))
        ps_proj = ctx.enter_context(tc.tile_pool(name="ps_proj", bufs=2, space="PSUM"))
        ps_sc = ctx.enter_context(tc.tile_pool(name="ps_sc", bufs=2, space="PSUM"))
        ps_tr = ctx.enter_context(tc.tile_pool(name="ps_tr", bufs=2, space="PSUM"))
        ps_a = ctx.enter_context(tc.tile_pool(name="ps_a", bufs=2, space="PSUM"))

        # constants
        masks_sb = persist.tile([128, 896], F32, name="masks_sb")
        nc.sync.dma_start(out=masks_sb, in_=dmask[:, :])
        ident_sb = persist.tile([128, 128], F16, name="ident_sb")
        nc.sync.dma_start(out=ident_sb, in_=dident[:, :])

        # persistent per-subgroup tensors
        kTh = [persist.tile([128, T], F16, name=f"kTh{h}") for h in range(SH)]
        kTl = [persist.tile([128, T], F16, name=f"kTl{h}") for h in range(SH)]
        v_sb = [persist.tile([128, GDIM], F16, name=f"v{s}") for s in range(NSB)]
        aT = [persist.tile([128, T], F16, name=f"aT{d}") for d in range(HPC)]

        def load_x_chunk(c):
            xh_c = xpool.tile([128, KT, TC], F16, name="xh_c")
            xl_c = xpool.tile([128, KT, TC], F16, name="xl_c")
            nc.sync.dma_start(out=xh_c, in_=_strided3(dxh[:, :], T, c * TC, 128, KT, TC, 128 * T))
            nc.sync.dma_start(out=xl_c, in_=_strided3(dxl[:, :], T, c * TC, 128, KT, TC, 128 * T))
            return xh_c, xl_c

        def load_w(dram, name, sub):
            t = wpool.tile([128, KT, GDIM], F16, name=name, tag=name)
            nc.sync.dma_start(
                out=t, in_=_strided3(dram[:, :], HPC * DH, sub * GDIM, 128, KT, GDIM, 128 * HPC * DH))
            return t

        for sub in range(NSUB):
            # ---- pass A: kT (hi/lo) and v for this subgroup's 4 heads ----
            wkh_sb = load_w(dwkh, "wAh", sub)
            wkl_sb = load_w(dwkl, "wAl", sub)
            wv_sb = wpool.tile([128, KT, GDIM], F16, name="wv_sb", tag="wvqc")
            nc.sync.dma_start(
                out=wv_sb, in_=_strided3(dwv[:, :], HPC * DH, sub * GDIM, 128, KT, GDIM, 128 * HPC * DH))
            for c in range(NCH):
                xh_c, xl_c = load_x_chunk(c)
                for h in range(SH):
                    psk = ps_proj.tile([128, TC], F32, name="psk", tag="proj")
                    hs = h * DH
                    for i in range(KT):
                        nc.tensor.matmul(psk, wkh_sb[:, i, hs:hs + DH], xh_c[:, i, :],
                                         start=(i == 0), stop=False)
                    for i in range(KT):
                        nc.tensor.matmul(psk, wkh_sb[:, i, hs:hs + DH], xl_c[:, i, :],
                                         start=False, stop=False)
                    for i in range(KT):
                        nc.tensor.matmul(psk, wkl_sb[:, i, hs:hs + DH], xh_c[:, i, :],
                                         start=False, stop=(i == KT - 1))
                    nc.scalar.copy(out=kTh[h][:, c * TC:(c + 1) * TC], in_=psk)
                    nc.vector.tensor_sub(out=kTl[h][:, c * TC:(c + 1) * TC],
                                         in0=psk, in1=kTh[h][:, c * TC:(c + 1) * TC])
                for sblk in range(4):
                    psv = ps_proj.tile([128, GDIM], F32, name="psv", tag="proj")
                    ss = sblk * 128
                    for i in range(KT):
                        nc.tensor.matmul(psv, xh_c[:, i, ss:ss + 128], wv_sb[:, i, :],
                                         start=(i == 0), stop=(i == KT - 1))
                    nc.scalar.copy(out=v_sb[c * 4 + sblk], in_=psv)

            # ---- pass B: q projection + attention ----
            wqh_sb = load_w(dwqh, "wAh", sub)
            wql_sb = load_w(dwql, "wAl", sub)
            for c in range(NCH):
                xh_c, xl_c = load_x_chunk(c)
                qc_all = wpool.tile([128, 2 * SH, TC], F16, name="qc_all", tag="wvqc")
                qch = [qc_all[:, h] for h in range(SH)]
                qcl = [qc_all[:, SH + h] for h in range(SH)]
                for h in range(SH):
                    psq = ps_proj.tile([128, TC], F32, name="psq", tag="proj")
                    hs = h * DH
                    for i in range(KT):
                        nc.tensor.matmul(psq, wqh_sb[:, i, hs:hs + DH], xh_c[:, i, :],
                                         start=(i == 0), stop=False)
                    for i in range(KT):
                        nc.tensor.matmul(psq, wqh_sb[:, i, hs:hs + DH], xl_c[:, i, :],
                                         start=False, stop=False)
                    for i in range(KT):
                        nc.tensor.matmul(psq, wql_sb[:, i, hs:hs + DH], xh_c[:, i, :],
                                         start=False, stop=(i == KT - 1))
                    nc.scalar.copy(out=qch[h], in_=psq)
                    nc.vector.tensor_sub(out=qcl[h], in0=psq, in1=qch[h])

                for h in range(SH):
                    pT = [ptp.tile([128, TC], F16, name=f"pT{sb}", tag=f"pT{sb}")
                          for sb in range(4 * c + 4)]
                    for tb_i in range(4):
                        tb = 4 * c + tb_i
                        s_len = (tb + 1) * 128
                        n_sc = tb // 4 + 1
                        ssb = work.tile([128, T], F32, name="ssb", tag="ssb", bufs=1)
                        p = work.tile([128, T], F16, name="p", tag="p", bufs=1)
                        acc = small.tile([128, 4], F32, name="acc")
                        negmax = small.tile([128, 1], F32, name="negmax")
                        ssum = small.tile([128, 1], F32, name="ssum")
                        rsum = small.tile([128, 1], F32, name="rsum")
                        diag = small.tile([128, 128], F16, name="diag")
                        ts0 = tb_i * 128
                        for sc in range(n_sc):
                            w = min(TC, s_len - sc * TC)
                            pss = ps_sc.tile([128, TC], F32, name="pss", tag="pssc")
                            nc.tensor.matmul(pss[:, :w], qch[h][:, ts0:ts0 + 128],
                                             kTh[h][:, sc * TC:sc * TC + w],
                                             start=True, stop=False)
                            nc.tensor.matmul(pss[:, :w], qch[h][:, ts0:ts0 + 128],
                                             kTl[h][:, sc * TC:sc * TC + w],
                                             start=False, stop=False)
                            nc.tensor.matmul(pss[:, :w], qcl[h][:, ts0:ts0 + 128],
                                             kTh[h][:, sc * TC:sc * TC + w],
                                             start=False, stop=True)
                            if sc == n_sc - 1:
                                mo = 384 - (tb % 4) * 128
                                nc.vector.tensor_add(out=ssb[:, sc * TC:sc * TC + w],
                                                     in0=pss[:, :w],
                                                     in1=masks_sb[:, mo:mo + w])
                            else:
                                nc.scalar.copy(out=ssb[:, sc * TC:sc * TC + w],
                                               in_=pss[:, :w])
                        nc.vector.reduce_max(out=negmax, in_=ssb[:, :s_len],
                                             axis=mybir.AxisListType.X, negate=True)
                        for sc in range(n_sc):
                            w = min(TC, s_len - sc * TC)
                            nc.scalar.activation(
                                out=p[:, sc * TC:sc * TC + w],
                                in_=ssb[:, sc * TC:sc * TC + w],
                                func=mybir.ActivationFunctionType.Exp,
                                bias=negmax[:, 0:1], scale=1.0,
                                accum_out=acc[:, sc:sc + 1])
                        nc.vector.reduce_sum(out=ssum, in_=acc[:, :n_sc],
                                             axis=mybir.AxisListType.X)
                        nc.vector.reciprocal(out=rsum, in_=ssum)
                        nc.vector.tensor_scalar_mul(diag, ident_sb, rsum[:, 0:1])
                        for sb in range(tb + 1):
                            pst = ps_tr.tile([128, 128], F32, name="pst", tag="pstr")
                            nc.tensor.matmul(pst, p[:, sb * 128:sb * 128 + 128], diag,
                                             start=True, stop=True)
                            if sb % 2 == 0:
                                nc.scalar.copy(out=pT[sb][:, ts0:ts0 + 128], in_=pst)
                            else:
                                nc.vector.tensor_copy(out=pT[sb][:, ts0:ts0 + 128],
                                                      in_=pst)
                    psa = ps_a.tile([128, TC], F32, name="psa", tag="psa")
                    hs = h * DH
                    for sb in range(4 * c + 4):
                        off = max(0, (sb - 4 * c) * 128)
                        nc.tensor.matmul(psa[:, off:TC], v_sb[sb][:, hs:hs + DH],
                                         pT[sb][:, off:TC],
                                         start=(sb == 0), stop=(sb == 4 * c + 3))
                    nc.scalar.copy(out=aT[sub * SH + h][:, c * TC:(c + 1) * TC],
                                   in_=psa)

        # ---- phase C: output projection (wo streamed per output chunk) ----
        for oc in range(NCH):
            wo_c = xpool.tile([128, HPC, TC], F16, name="wo_c", tag="xh_c")
            nc.sync.dma_start(
                out=wo_c, in_=_strided3(dwo[:, :], DOUT, oc * TC, 128, HPC, TC, 128 * DOUT))
            for tb in range(NSB):
                pso = ps_proj.tile([128, TC], F32, name="pso", tag="proj")
                for d in range(HPC):
                    nc.tensor.matmul(pso, aT[d][:, tb * 128:(tb + 1) * 128],
                                     wo_c[:, d, :],
                                     start=(d == 0), stop=(d == HPC - 1))
                ob = work.tile([128, TC], F32, name="ob", tag="ob")
                nc.vector.tensor_copy(out=ob, in_=pso)
                tgt = dpart if use_collective else dout
                nc.sync.dma_start(
                    out=tgt[tb * 128:(tb + 1) * 128, oc * TC:(oc + 1) * TC], in_=ob)

        if use_collective:
            nc.gpsimd.collective_compute(
                "AllReduce", mybir.AluOpType.add,
                replica_groups=[[0, 1], [2, 3], [4, 5], [6, 7]],
                ins=[dpart[:, :]], outs=[darout[:, :]])
            nc.sync.dma_start(out=dout[:, :], in_=darout[:, :])

    nc.compile()
    return nc


def _split16(a):
    hi = a.astype(np.float16)
    lo = (a - hi.astype(np.float32)).astype(np.float16)
    return hi, lo


def _prep_inputs(x, Wq, Wk, Wv, Wo):
    x = np.asarray(x, dtype=np.float32)
    Wq = np.asarray(Wq, dtype=np.float32)
    Wk = np.asarray(Wk, dtype=np.float32)
    Wv = np.asarray(Wv, dtype=np.float32)
    Wo = np.asarray(Wo, dtype=np.float32)

    masks = np.zeros((128, 896), np.float32)
    t_i = np.arange(128)[:, None]
    u_i = np.arange(896)[None, :]
    masks[u_i > 384 + t_i] = -1.0e30
    ident = np.eye(128, dtype=np.float16)

    Wq_s = Wq * np.float32(INV_SCALE)

    in_maps = []
    for c in range(8):
        b, g = c // 2, c % 2
        cols = slice(g * HPC * DH, (g + 1) * HPC * DH)
        xT = np.ascontiguousarray(x[b].T)
        xh, xl = _split16(xT)
        wqh, wql = _split16(np.ascontiguousarray(Wq_s[:, cols]))
        wkh, wkl = _split16(np.ascontiguousarray(Wk[:, cols]))
        wv = np.ascontiguousarray(Wv[:, cols]).astype(np.float16)
        wo = np.ascontiguousarray(Wo[cols, :]).astype(np.float16)
        in_maps.append({
            "xh": xh, "xl": xl, "wqh": wqh, "wql": wql, "wkh": wkh,
            "wkl": wkl, "wv": wv, "wo": wo, "masks": masks, "ident": ident,
        })
    return in_maps


def run(x, Wq, Wk, Wv, Wo, trace=False, **kw):
    if "nc" not in _cached:
        _cached["nc"] = build_nc()
    nc = _cached["nc"]
    in_maps = _prep_inputs(x, Wq, Wk, Wv, Wo)
    res = run_bass_kernel_spmd(nc, in_maps, core_ids=list(range(8)), trace=trace, **kw)
    if USE_COLLECTIVE:
        out = np.stack([res.results[2 * b]["out"] for b in range(B)])
    else:
        out = np.stack([res.results[2 * b]["out"] + res.results[2 * b + 1]["out"]
                        for b in range(B)])
    return out.astype(np.float32), res


def kernel(x, Wq, Wk, Wv, Wo):
    out, _ = run(x, Wq, Wk, Wv, Wo)
    return out


# revision 4
# speedup vs baseline: 1.0564x; 1.0564x over previous
#!/usr/bin/env python3
"""MultiHeadCausalAttention on 8 trn2 NeuronCores.

Sharding: core c handles batch b = c//2 and head-group g = c%2 (8 of 16 heads,
Megatron-style column shard of Wq/Wk/Wv, row shard of Wo). The pair (2b, 2b+1)
all-reduces its partial output projection on-device.

Numerics: logit path (q, k projections and q.k^T scores) uses fp16 hi/lo
split operands with 3 accumulating matmuls per product (fp32-grade results at
full PE rate; measured ~3e-7 rel err). Value path (v, attn@v, Wo) uses single
fp16 (~3e-4 rel err). The softmax scale sqrt(2048) is folded into Wq on host.
Softmax: row max via DVE reduce (negated -> exp bias), exp on ACT with
accum_out row sums; normalization is folded into the p^T transpose by
multiplying with diag(1/sum) on the PE.
"""
import sys
sys.path.insert(0, "/opt/trn_rl_repo")

import numpy as np
from contextlib import ExitStack

import concourse.bass as bass
import concourse.bacc as bacc
import concourse.mybir as mybir
import concourse.tile as tile
from concourse.bass_utils import run_bass_kernel_spmd

F16 = mybir.dt.float16
F32 = mybir.dt.float32

B, T, DIN, DOUT = 4, 2048, 2048, 2048
H, DH = 16, 128
INV_SCALE = float(DOUT) ** 0.5
HPC = 8            # heads per core
SH = 2             # heads per subgroup
NSUB = HPC // SH   # subgroups
TC = 512           # t/s chunk width
NCH = T // TC      # 4 chunks
KT = DIN // 128    # 16 contraction tiles
NSB = T // 128     # 16 s-blocks
GDIM = SH * DH     # head-dim cols per subgroup

USE_COLLECTIVE = True

_cached = {}


def _strided3(dram_ap, row_stride, col_off, n_p, n_k, n_w, k_stride):
    """AP over a 2-D DRAM tensor: [partition(n_p rows), k(n_k row-tiles), w cols]."""
    return bass.AP(tensor=dram_ap.tensor, offset=dram_ap.offset + col_off,
                   ap=[[row_stride, n_p], [k_stride, n_k], [1, n_w]])


def build_nc(use_collective=USE_COLLECTIVE):
    nc = bacc.Bacc()

    dxh = nc.dram_tensor("xh", [DIN, T], F16, kind="ExternalInput")
    dxl = nc.dram_tensor("xl", [DIN, T], F16, kind="ExternalInput")
    dwqh = nc.dram_tensor("wqh", [DIN, HPC * DH], F16, kind="ExternalInput")
    dwql = nc.dram_tensor("wql", [DIN, HPC * DH], F16, kind="ExternalInput")
    dwkh = nc.dram_tensor("wkh", [DIN, HPC * DH], F16, kind="ExternalInput")
    dwkl = nc.dram_tensor("wkl", [DIN, HPC * DH], F16, kind="ExternalInput")
    dwv = nc.dram_tensor("wv", [DIN, HPC * DH], F16, kind="ExternalInput")
    dwo = nc.dram_tensor("wo", [HPC * DH, DOUT], F16, kind="ExternalInput")
    dmask = nc.dram_tensor("masks", [128, 896], F32, kind="ExternalInput")
    dident = nc.dram_tensor("ident", [128, 128], F16, kind="ExternalInput")

    dout = nc.dram_tensor("out", [T, DOUT], F32, kind="ExternalOutput")
    if use_collective:
        dpart = nc.dram_tensor("part", [T, DOUT], F32, kind="Internal")
        darout = nc.dram_tensor("arout", [T, DOUT], F32, kind="Internal")
    else:
        dpart = None
        darout = None

    with tile.TileContext(nc) as tc, ExitStack() as ctx:
        persist = ctx.enter_context(tc.tile_pool(name="persist", bufs=1))
        wpool = ctx.enter_context(tc.tile_pool(name="wpool", bufs=2))
        xpool = ctx.enter_context(tc.tile_pool(name="xpool", bufs=1))
        work = ctx.enter_context(tc.tile_pool(name="work", bufs=2))
        ptp = ctx.enter_context(tc.tile_pool(name="ptp", bufs=2))
        small = ctx.enter_context(tc.tile_pool(name="small", bufs=4))
        ps_proj = ctx.enter_context(tc.tile_pool(name="ps_proj", bufs=2, space="PSUM"))
        ps_sc = ctx.enter_context(tc.tile_pool(name="ps_sc", bufs=2, space="PSUM"))
        ps_tr = ctx.enter_context(tc.tile_pool(name="ps_tr", bufs=2, space="PSUM"))
        ps_a = ctx.enter_context(tc.tile_pool(name="ps_a", bufs=2, space="PSUM"))

        # constants
        masks_sb = persist.tile([128, 896], F32, name="masks_sb")
        nc.sync.dma_start(out=masks_sb, in_=dmask[:, :])
        ident_sb = persist.tile([128, 128], F16, name="ident_sb")
        nc.sync.dma_start(out=ident_sb, in_=dident[:, :])

        # persistent per-subgroup tensors
        kTh = [persist.tile([128, T], F16, name=f"kTh{h}") for h in range(SH)]
        kTl = [persist.tile([128, T], F16, name=f"kTl{h}") for h in range(SH)]
        v_sb = [persist.tile([128, GDIM], F16, name=f"v{s}") for s in range(NSB)]
        aT = [persist.tile([128, T], F16, name=f"aT{d}") for d in range(HPC)]

        def load_x_chunk(c):
            xh_c = xpool.tile([128, KT, TC], F16, name="xh_c")
            xl_c = xpool.tile([128, KT, TC], F16, name="xl_c")
            nc.sync.dma_start(out=xh_c, in_=_strided3(dxh[:, :], T, c * TC, 128, KT, TC, 128 * T))
            nc.sync.dma_start(out=xl_c, in_=_strided3(dxl[:, :], T, c * TC, 128, KT, TC, 128 * T))
            return xh_c, xl_c

        def load_w(dram, name, tag, sub):
            t = wpool.tile([128, KT, GDIM], F16, name=name, tag=tag)
            nc.sync.dma_start(
                out=t, in_=_strided3(dram[:, :], HPC * DH, sub * GDIM, 128, KT, GDIM,
                                     128 * HPC * DH))
            return t

        for sub in range(NSUB):
            # ---- pass A: kT (hi/lo) and v for this subgroup's heads ----
            wkh_sb = load_w(dwkh, "wkh_sb", "wAh", sub)
            wkl_sb = load_w(dwkl, "wkl_sb", "wAl", sub)
            wv_sb = load_w(dwv, "wv_sb", "wvqc", sub)
            for c in range(NCH):
                xh_c, xl_c = load_x_chunk(c)
                for h in range(SH):
                    psk = ps_proj.tile([128, TC], F32, name="psk", tag="proj")
                    hs = h * DH
                    for i in range(KT):
                        nc.tensor.matmul(psk, wkh_sb[:, i, hs:hs + DH], xh_c[:, i, :],
                                         start=(i == 0), stop=False)
                    for i in range(KT):
                        nc.tensor.matmul(psk, wkh_sb[:, i, hs:hs + DH], xl_c[:, i, :],
                                         start=False, stop=False)
                    for i in range(KT):
                        nc.tensor.matmul(psk, wkl_sb[:, i, hs:hs + DH], xh_c[:, i, :],
                                         start=False, stop=(i == KT - 1))
                    nc.scalar.copy(out=kTh[h][:, c * TC:(c + 1) * TC], in_=psk)
                    nc.vector.tensor_sub(out=kTl[h][:, c * TC:(c + 1) * TC],
                                         in0=psk, in1=kTh[h][:, c * TC:(c + 1) * TC])
                for sblk in range(4):
                    psv = ps_proj.tile([128, GDIM], F32, name="psv", tag="proj")
                    ss = sblk * 128
                    for i in range(KT):
                        nc.tensor.matmul(psv, xh_c[:, i, ss:ss + 128], wv_sb[:, i, :],
                                         start=(i == 0), stop=(i == KT - 1))
                    nc.scalar.copy(out=v_sb[c * 4 + sblk], in_=psv)

            # ---- pass B: q projection + attention ----
            wqh_sb = load_w(dwqh, "wqh_sb", "wAh", sub)
            wql_sb = load_w(dwql, "wql_sb", "wAl", sub)
            for c in range(NCH):
                xh_c, xl_c = load_x_chunk(c)
                qc_all = wpool.tile([128, 2 * SH, TC], F16, name="qc_all", tag="wvqc")
                qch = [qc_all[:, h] for h in range(SH)]
                qcl = [qc_all[:, SH + h] for h in range(SH)]
                for h in range(SH):
                    psq = ps_proj.tile([128, TC], F32, name="psq", tag="proj")
                    hs = h * DH
                    for i in range(KT):
                        nc.tensor.matmul(psq, wqh_sb[:, i, hs:hs + DH], xh_c[:, i, :],
                                         start=(i == 0), stop=False)
                    for i in range(KT):
                        nc.tensor.matmul(psq, wqh_sb[:, i, hs:hs + DH], xl_c[:, i, :],
                                         start=False, stop=False)
                    for i in range(KT):
                        nc.tensor.matmul(psq, wql_sb[:, i, hs:hs + DH], xh_c[:, i, :],
                                         start=False, stop=(i == KT - 1))
                    nc.scalar.copy(out=qch[h], in_=psq)
                    nc.vector.tensor_sub(out=qcl[h], in0=psq, in1=qch[h])

                for h in range(SH):
                    pT = [ptp.tile([128, TC], F16, name=f"pT{sb}", tag=f"pT{sb}")
                          for sb in range(4 * c + 4)]
                    for tb_i in range(4):
                        tb = 4 * c + tb_i
                        s_len = (tb + 1) * 128
                        n_sc = tb // 4 + 1
                        ssb = work.tile([128, T], F32, name="ssb", tag="ssb")
                        p = work.tile([128, T], F16, name="p", tag="p")
                        acc = small.tile([128, 4], F32, name="acc")
                        negmax = small.tile([128, 1], F32, name="negmax")
                        ssum = small.tile([128, 1], F32, name="ssum")
                        rsum = small.tile([128, 1], F32, name="rsum")
                        diag = small.tile([128, 128], F16, name="diag")
                        ts0 = tb_i * 128
                        for sc in range(n_sc):
                            w = min(TC, s_len - sc * TC)
                            pss = ps_sc.tile([128, TC], F32, name="pss", tag="pssc")
                            nc.tensor.matmul(pss[:, :w], qch[h][:, ts0:ts0 + 128],
                                             kTh[h][:, sc * TC:sc * TC + w],
                                             start=True, stop=False)
                            nc.tensor.matmul(pss[:, :w], qch[h][:, ts0:ts0 + 128],
                                             kTl[h][:, sc * TC:sc * TC + w],
                                             start=False, stop=False)
                            nc.tensor.matmul(pss[:, :w], qcl[h][:, ts0:ts0 + 128],
                                             kTh[h][:, sc * TC:sc * TC + w],
                                             start=False, stop=True)
                            if sc == n_sc - 1:
                                mo = 384 - (tb % 4) * 128
                                nc.vector.tensor_add(out=ssb[:, sc * TC:sc * TC + w],
                                                     in0=pss[:, :w],
                                                     in1=masks_sb[:, mo:mo + w])
                            else:
                                nc.scalar.copy(out=ssb[:, sc * TC:sc * TC + w],
                                               in_=pss[:, :w])
                        nc.vector.reduce_max(out=negmax, in_=ssb[:, :s_len],
                                             axis=mybir.AxisListType.X, negate=True)
                        for sc in range(n_sc):
                            w = min(TC, s_len - sc * TC)
                            nc.scalar.activation(
                                out=p[:, sc * TC:sc * TC + w],
                                in_=ssb[:, sc * TC:sc * TC + w],
                                func=mybir.ActivationFunctionType.Exp,
                                bias=negmax[:, 0:1], scale=1.0,
                                accum_out=acc[:, sc:sc + 1])
                        nc.vector.reduce_sum(out=ssum, in_=acc[:, :n_sc],
                                             axis=mybir.AxisListType.X)
                        nc.vector.reciprocal(out=rsum, in_=ssum)
                        nc.vector.tensor_scalar_mul(diag, ident_sb, rsum[:, 0:1])
                        for sb in range(tb + 1):
                            pst = ps_tr.tile([128, 128], F32, name="pst", tag="pstr")
                            nc.tensor.matmul(pst, p[:, sb * 128:sb * 128 + 128], diag,
                                             start=True, stop=True)
                            if sb % 2 == 0:
                                nc.scalar.copy(out=pT[sb][:, ts0:ts0 + 128], in_=pst)
                            else:
                                nc.vector.tensor_copy(out=pT[sb][:, ts0:ts0 + 128],
                                                      in_=pst)
                    psa = ps_a.tile([128, TC], F32, name="psa", tag="psa")
                    hs = h * DH
                    for sb in range(4 * c + 4):
                        off = max(0, (sb - 4 * c) * 128)
                        nc.tensor.matmul(psa[:, off:TC], v_sb[sb][:, hs:hs + DH],
                                         pT[sb][:, off:TC],
                                         start=(sb == 0), stop=(sb == 4 * c + 3))
                    nc.scalar.copy(out=aT[sub * SH + h][:, c * TC:(c + 1) * TC],
                                   in_=psa)

        # ---- phase C: output projection (wo streamed per output chunk) ----
        for oc in range(NCH):
            wo_c = xpool.tile([128, HPC, TC], F16, name="wo_c", tag="xh_c")
            nc.sync.dma_start(
                out=wo_c, in_=_strided3(dwo[:, :], DOUT, oc * TC, 128, HPC, TC, 128 * DOUT))
            for tb in range(NSB):
                pso = ps_proj.tile([128, TC], F32, name="pso", tag="proj")
                for d in range(HPC):
                    nc.tensor.matmul(pso, aT[d][:, tb * 128:(tb + 1) * 128],
                                     wo_c[:, d, :],
                                     start=(d == 0), stop=(d == HPC - 1))
                ob = work.tile([128, TC], F32, name="ob", tag="ob")
                nc.vector.tensor_copy(out=ob, in_=pso)
                tgt = dpart if use_collective else dout
                nc.sync.dma_start(
                    out=tgt[tb * 128:(tb + 1) * 128, oc * TC:(oc + 1) * TC], in_=ob)

        if use_collective:
            nc.gpsimd.collective_compute(
                "AllReduce", mybir.AluOpType.add,
                replica_groups=[[0, 1], [2, 3], [4, 5], [6, 7]],
                ins=[dpart[:, :]], outs=[darout[:, :]])
            nc.sync.dma_start(out=dout[:, :], in_=darout[:, :])

    nc.compile()
    return nc


def _split16(a):
    hi = a.astype(np.float16)
    lo = (a - hi.astype(np.float32)).astype(np.float16)
    return hi, lo


def _prep_inputs(x, Wq, Wk, Wv, Wo):
    x = np.asarray(x, dtype=np.float32)
    Wq = np.asarray(Wq, dtype=np.float32)
    Wk = np.asarray(Wk, dtype=np.float32)
    Wv = np.asarray(Wv, dtype=np.float32)
    Wo = np.asarray(Wo, dtype=np.float32)

    masks = np.zeros((128, 896), np.float32)
    t_i = np.arange(128)[:, None]
    u_i = np.arange(896)[None, :]
    masks[u_i > 384 + t_i] = -1.0e30
    ident = np.eye(128, dtype=np.float16)

    Wq_s = Wq * np.float32(INV_SCALE)

    in_maps = []
    for c in range(8):
        b, g = c // 2, c % 2
        cols = slice(g * HPC * DH, (g + 1) * HPC * DH)
        xT = np.ascontiguousarray(x[b].T)
        xh, xl = _split16(xT)
        wqh, wql = _split16(np.ascontiguousarray(Wq_s[:, cols]))
        wkh, wkl = _split16(np.ascontiguousarray(Wk[:, cols]))
        wv = np.ascontiguousarray(Wv[:, cols]).astype(np.float16)
        wo = np.ascontiguousarray(Wo[cols, :]).astype(np.float16)
        in_maps.append({
            "xh": xh, "xl": xl, "wqh": wqh, "wql": wql, "wkh": wkh,
            "wkl": wkl, "wv": wv, "wo": wo, "masks": masks, "ident": ident,
        })
    return in_maps


def run(x, Wq, Wk, Wv, Wo, trace=False, **kw):
    if "nc" not in _cached:
        _cached["nc"] = build_nc()
    nc = _cached["nc"]
    in_maps = _prep_inputs(x, Wq, Wk, Wv, Wo)
    res = run_bass_kernel_spmd(nc, in_maps, core_ids=list(range(8)), trace=trace, **kw)
    if USE_COLLECTIVE:
        out = np.stack([res.results[2 * b]["out"] for b in range(B)])
    else:
        out = np.stack([res.results[2 * b]["out"] + res.results[2 * b + 1]["out"]
                        for b in range(B)])
    return out.astype(np.float32), res


def kernel(x, Wq, Wk, Wv, Wo):
    out, _ = run(x, Wq, Wk, Wv, Wo)
    return out


# revision 5
# speedup vs baseline: 1.1799x; 1.1170x over previous
#!/usr/bin/env python3
"""MultiHeadCausalAttention on 8 trn2 NeuronCores.

Sharding: core c handles batch b = c//2 and head-group g = c%2 (8 of 16 heads,
Megatron-style column shard of Wq/Wk/Wv, row shard of Wo). The pair (2b, 2b+1)
all-reduces its partial output projection on-device.

Numerics: logit path (q, k projections and q.k^T scores) uses fp16 hi/lo
split operands with 3 accumulating matmuls per product (fp32-grade results at
full PE rate; measured ~3e-7 rel err). Value path (v, attn@v, Wo) uses single
fp16 (~3e-4 rel err). The softmax scale sqrt(2048) is folded into Wq on host.
Softmax: scores stay in PSUM; diagonal blocks get the causal mask added
in-place (DVE); row max via per-chunk DVE reduce partials (negated -> exp
bias); exp on ACT reads PSUM, writes fp16 p + accum_out row sums;
normalization is folded into the p^T transpose by multiplying with
diag(1/sum) on the PE. attn@v accumulates ragged-width matmuls into aT[d,t],
which feeds Wo directly as lhsT (no extra transpose).

All inputs are host-pretiled so every DMA reads fully contiguous
per-partition rows.
"""
import sys
sys.path.insert(0, "/opt/trn_rl_repo")

import numpy as np
from contextlib import ExitStack

import concourse.bass as bass
import concourse.bacc as bacc
import concourse.mybir as mybir
import concourse.tile as tile
from concourse.bass_utils import run_bass_kernel_spmd

F16 = mybir.dt.float16
F32 = mybir.dt.float32

B, T, DIN, DOUT = 4, 2048, 2048, 2048
H, DH = 16, 128
INV_SCALE = float(DOUT) ** 0.5
HPC = 8            # heads per core
SH = 2             # heads per subgroup
NSUB = HPC // SH   # subgroups
TC = 512           # t/s chunk width
NCH = T // TC      # 4 chunks
KT = DIN // 128    # 16 contraction tiles
NSB = T // 128     # 16 s-blocks
GDIM = SH * DH     # head-dim cols per subgroup

USE_COLLECTIVE = True

_cached = {}


def build_nc(use_collective=USE_COLLECTIVE):
    nc = bacc.Bacc()

    # pretiled inputs: x* [NCH, 128, KT, TC]; w* [NSUB, 128, KT, GDIM];
    # wo [NCH, 128, HPC, TC]
    dxh = nc.dram_tensor("xh", [NCH, 128, KT, TC], F16, kind="ExternalInput")
    dxl = nc.dram_tensor("xl", [NCH, 128, KT, TC], F16, kind="ExternalInput")
    dwqh = nc.dram_tensor("wqh", [NSUB, 128, KT, GDIM], F16, kind="ExternalInput")
    dwql = nc.dram_tensor("wql", [NSUB, 128, KT, GDIM], F16, kind="ExternalInput")
    dwkh = nc.dram_tensor("wkh", [NSUB, 128, KT, GDIM], F16, kind="ExternalInput")
    dwkl = nc.dram_tensor("wkl", [NSUB, 128, KT, GDIM], F16, kind="ExternalInput")
    dwv = nc.dram_tensor("wv", [NSUB, 128, KT, GDIM], F16, kind="ExternalInput")
    dwo = nc.dram_tensor("wo", [NCH, 128, HPC, TC], F16, kind="ExternalInput")
    dmask = nc.dram_tensor("masks", [128, 896], F32, kind="ExternalInput")
    dident = nc.dram_tensor("ident", [128, 128], F16, kind="ExternalInput")

    dout = nc.dram_tensor("out", [T, DOUT], F32, kind="ExternalOutput")
    if use_collective:
        dpart = nc.dram_tensor("part", [T, DOUT], F32, kind="Internal")
        darout = nc.dram_tensor("arout", [T, DOUT], F32, kind="Internal")
    else:
        dpart = None
        darout = None

    with tile.TileContext(nc) as tc, ExitStack() as ctx:
        persist = ctx.enter_context(tc.tile_pool(name="persist", bufs=1))
        wpool = ctx.enter_context(tc.tile_pool(name="wpool", bufs=1))
        xpool = ctx.enter_context(tc.tile_pool(name="xpool", bufs=1))
        work = ctx.enter_context(tc.tile_pool(name="work", bufs=2))
        ptp = ctx.enter_context(tc.tile_pool(name="ptp", bufs=2))
        small = ctx.enter_context(tc.tile_pool(name="small", bufs=4))
        # "proj" psums are shared by q/k/v projections AND score chunks (up to
        # 4 score chunks alive at once for the last row block); pstr for
        # transposes, psa for attn@v accumulation.
        ps_proj = ctx.enter_context(tc.tile_pool(name="ps_proj", bufs=4, space="PSUM"))
        ps_tr = ctx.enter_context(tc.tile_pool(name="ps_tr", bufs=2, space="PSUM"))
        ps_a = ctx.enter_context(tc.tile_pool(name="ps_a", bufs=2, space="PSUM"))

        # constants
        masks_sb = persist.tile([128, 896], F32, name="masks_sb")
        nc.sync.dma_start(out=masks_sb, in_=dmask[:, :])
        ident_sb = persist.tile([128, 128], F16, name="ident_sb")
        nc.sync.dma_start(out=ident_sb, in_=dident[:, :])

        # persistent per-subgroup tensors
        kTh = [persist.tile([128, T], F16, name=f"kTh{h}") for h in range(SH)]
        kTl = [persist.tile([128, T], F16, name=f"kTl{h}") for h in range(SH)]
        v_sb = [persist.tile([128, GDIM], F16, name=f"v{s}") for s in range(NSB)]
        aT = [persist.tile([128, T], F16, name=f"aT{d}") for d in range(HPC)]

        def load_x_chunk(c):
            xh_c = xpool.tile([128, KT, TC], F16, name="xh_c")
            xl_c = xpool.tile([128, KT, TC], F16, name="xl_c")
            for k0 in range(0, KT, 4):  # split for pipelining w/ first matmuls
                nc.sync.dma_start(out=xh_c[:, k0:k0 + 4, :], in_=dxh[c, :, k0:k0 + 4, :])
                nc.sync.dma_start(out=xl_c[:, k0:k0 + 4, :], in_=dxl[c, :, k0:k0 + 4, :])
            return xh_c, xl_c

        def load_w(dram, name, sub, bufs=1):
            t = wpool.tile([128, KT, GDIM], F16, name=name, tag=name, bufs=bufs)
            nc.sync.dma_start(out=t, in_=dram[sub])
            return t

        def hiprec_proj(ps, wh, wl, xh_c, xl_c, hs):
            for i in range(KT):
                nc.tensor.matmul(ps, wh[:, i, hs:hs + DH], xh_c[:, i, :],
                                 start=(i == 0), stop=False)
            for i in range(KT):
                nc.tensor.matmul(ps, wh[:, i, hs:hs + DH], xl_c[:, i, :],
                                 start=False, stop=False)
            for i in range(KT):
                nc.tensor.matmul(ps, wl[:, i, hs:hs + DH], xh_c[:, i, :],
                                 start=False, stop=(i == KT - 1))

        for sub in range(NSUB):
            wkh_sb = load_w(dwkh, "wkh_sb", sub, bufs=2)
            wkl_sb = load_w(dwkl, "wkl_sb", sub)
            wv_sb = load_w(dwv, "wv_sb", sub)
            wqh_sb = load_w(dwqh, "wqh_sb", sub, bufs=2)
            wql_sb = load_w(dwql, "wql_sb", sub)
            for c in range(NCH):
                xh_c, xl_c = load_x_chunk(c)

                # k projection (hi/lo) for this chunk's s columns
                for h in range(SH):
                    psk = ps_proj.tile([128, TC], F32, name="psk", tag="proj")
                    hiprec_proj(psk, wkh_sb, wkl_sb, xh_c, xl_c, h * DH)
                    nc.scalar.copy(out=kTh[h][:, c * TC:(c + 1) * TC], in_=psk)
                    nc.vector.tensor_sub(out=kTl[h][:, c * TC:(c + 1) * TC],
                                         in0=psk, in1=kTh[h][:, c * TC:(c + 1) * TC])
                # v for this chunk's s rows
                for sblk in range(4):
                    psv = ps_proj.tile([128, GDIM], F32, name="psv", tag="proj")
                    ss = sblk * 128
                    for i in range(KT):
                        nc.tensor.matmul(psv, xh_c[:, i, ss:ss + 128], wv_sb[:, i, :],
                                         start=(i == 0), stop=(i == KT - 1))
                    nc.scalar.copy(out=v_sb[c * 4 + sblk], in_=psv)
                # q (hi/lo) for this chunk's t columns
                qc_all = work.tile([128, 2 * SH, TC], F16, name="qc_all", tag="qc")
                qch = [qc_all[:, h] for h in range(SH)]
                qcl = [qc_all[:, SH + h] for h in range(SH)]
                for h in range(SH):
                    psq = ps_proj.tile([128, TC], F32, name="psq", tag="proj")
                    hiprec_proj(psq, wqh_sb, wql_sb, xh_c, xl_c, h * DH)
                    nc.scalar.copy(out=qch[h], in_=psq)
                    nc.vector.tensor_sub(out=qcl[h], in0=psq, in1=qch[h])

                # attention for this chunk's 4 row blocks
                for h in range(SH):
                    pT = [ptp.tile([128, TC], F16, name=f"pT{sb}", tag=f"pT{sb}")
                          for sb in range(4 * c + 4)]
                    for tb_i in range(4):
                        tb = 4 * c + tb_i
                        s_len = (tb + 1) * 128
                        n_sc = tb // 4 + 1
                        p = work.tile([128, T], F16, name="p", tag="p")
                        acc = small.tile([128, 4], F32, name="acc")
                        mstat = small.tile([128, 4], F32, name="mstat")
                        negmax = small.tile([128, 1], F32, name="negmax")
                        ssum = small.tile([128, 1], F32, name="ssum")
                        rsum = small.tile([128, 1], F32, name="rsum")
                        diag = small.tile([128, 128], F16, name="diag")
                        ts0 = tb_i * 128
                        pss_l = []
                        for sc in range(n_sc):
                            w = min(TC, s_len - sc * TC)
                            pss = ps_proj.tile([128, TC], F32, name="pss", tag="proj")
                            pss_l.append(pss)
                            nc.tensor.matmul(pss[:, :w], qch[h][:, ts0:ts0 + 128],
                                             kTh[h][:, sc * TC:sc * TC + w],
                                             start=True, stop=False)
                            nc.tensor.matmul(pss[:, :w], qch[h][:, ts0:ts0 + 128],
                                             kTl[h][:, sc * TC:sc * TC + w],
                                             start=False, stop=False)
                            nc.tensor.matmul(pss[:, :w], qcl[h][:, ts0:ts0 + 128],
                                             kTh[h][:, sc * TC:sc * TC + w],
                                             start=False, stop=True)
                            if sc == n_sc - 1:
                                mo = 384 - (tb % 4) * 128
                                nc.vector.tensor_add(out=pss[:, :w], in0=pss[:, :w],
                                                     in1=masks_sb[:, mo:mo + w])
                            nc.vector.reduce_max(out=mstat[:, sc:sc + 1],
                                                 in_=pss[:, :w],
                                                 axis=mybir.AxisListType.X)
                        nc.vector.reduce_max(out=negmax, in_=mstat[:, :n_sc],
                                             axis=mybir.AxisListType.X, negate=True)
                        for sc in range(n_sc):
                            w = min(TC, s_len - sc * TC)
                            nc.scalar.activation(
                                out=p[:, sc * TC:sc * TC + w],
                                in_=pss_l[sc][:, :w],
                                func=mybir.ActivationFunctionType.Exp,
                                bias=negmax[:, 0:1], scale=1.0,
                                accum_out=acc[:, sc:sc + 1])
                        nc.vector.reduce_sum(out=ssum, in_=acc[:, :n_sc],
                                             axis=mybir.AxisListType.X)
                        nc.vector.reciprocal(out=rsum, in_=ssum)
                        nc.vector.tensor_scalar_mul(diag, ident_sb, rsum[:, 0:1])
                        for sb in range(tb + 1):
                            pst = ps_tr.tile([128, 128], F32, name="pst", tag="pstr")
                            nc.tensor.matmul(pst, p[:, sb * 128:sb * 128 + 128], diag,
                                             start=True, stop=True)
                            if sb % 2 == 0:
                                nc.scalar.copy(out=pT[sb][:, ts0:ts0 + 128], in_=pst)
                            else:
                                nc.vector.tensor_copy(out=pT[sb][:, ts0:ts0 + 128],
                                                      in_=pst)
                    psa = ps_a.tile([128, TC], F32, name="psa", tag="psa")
                    hs = h * DH
                    for sb in range(4 * c + 4):
                        off = max(0, (sb - 4 * c) * 128)
                        nc.tensor.matmul(psa[:, off:TC], v_sb[sb][:, hs:hs + DH],
                                         pT[sb][:, off:TC],
                                         start=(sb == 0), stop=(sb == 4 * c + 3))
                    nc.scalar.copy(out=aT[sub * SH + h][:, c * TC:(c + 1) * TC],
                                   in_=psa)

        # ---- phase C: output projection (wo streamed per output chunk) ----
        for oc in range(NCH):
            wo_c = xpool.tile([128, HPC, TC], F16, name="wo_c", tag="xh_c")
            nc.sync.dma_start(out=wo_c, in_=dwo[oc])
            for tb in range(NSB):
                pso = ps_proj.tile([128, TC], F32, name="pso", tag="proj")
                for d in range(HPC):
                    nc.tensor.matmul(pso, aT[d][:, tb * 128:(tb + 1) * 128],
                                     wo_c[:, d, :],
                                     start=(d == 0), stop=(d == HPC - 1))
                ob = work.tile([128, TC], F32, name="ob", tag="ob")
                nc.vector.tensor_copy(out=ob, in_=pso)
                tgt = dpart if use_collective else dout
                nc.sync.dma_start(
                    out=tgt[tb * 128:(tb + 1) * 128, oc * TC:(oc + 1) * TC], in_=ob)

        if use_collective:
            nc.gpsimd.collective_compute(
                "AllReduce", mybir.AluOpType.add,
                replica_groups=[[0, 1], [2, 3], [4, 5], [6, 7]],
                ins=[dpart[:, :]], outs=[darout[:, :]])
            nc.sync.dma_start(out=dout[:, :], in_=darout[:, :])

    nc.compile()
    return nc


def _split16(a):
    hi = a.astype(np.float16)
    lo = (a - hi.astype(np.float32)).astype(np.float16)
    return hi, lo


def _tile_rows(a, n_outer, width):
    """[R, C] -> [n_outer, 128, R//128, width] where C = n_outer*width."""
    r = a.shape[0]
    return np.ascontiguousarray(
        a.reshape(r // 128, 128, n_outer, width).transpose(2, 1, 0, 3))


def _prep_inputs(x, Wq, Wk, Wv, Wo):
    x = np.asarray(x, dtype=np.float32)
    Wq = np.asarray(Wq, dtype=np.float32)
    Wk = np.asarray(Wk, dtype=np.float32)
    Wv = np.asarray(Wv, dtype=np.float32)
    Wo = np.asarray(Wo, dtype=np.float32)

    masks = np.zeros((128, 896), np.float32)
    t_i = np.arange(128)[:, None]
    u_i = np.arange(896)[None, :]
    masks[u_i > 384 + t_i] = -1.0e30
    ident = np.eye(128, dtype=np.float16)

    Wq_s = Wq * np.float32(INV_SCALE)

    # per-batch x prep (shared by core pairs)
    xprep = []
    for b in range(B):
        xh, xl = _split16(np.ascontiguousarray(x[b].T))
        xprep.append((_tile_rows(xh, NCH, TC), _tile_rows(xl, NCH, TC)))
    # per-head-group weight prep
    wprep = []
    for g in range(2):
        cols = slice(g * HPC * DH, (g + 1) * HPC * DH)
        wqh, wql = _split16(np.ascontiguousarray(Wq_s[:, cols]))
        wkh, wkl = _split16(np.ascontiguousarray(Wk[:, cols]))
        wv = np.ascontiguousarray(Wv[:, cols]).astype(np.float16)
        wo = np.ascontiguousarray(Wo[cols, :]).astype(np.float16)
        wprep.append({
            "wqh": _tile_rows(wqh, NSUB, GDIM), "wql": _tile_rows(wql, NSUB, GDIM),
            "wkh": _tile_rows(wkh, NSUB, GDIM), "wkl": _tile_rows(wkl, NSUB, GDIM),
            "wv": _tile_rows(wv, NSUB, GDIM), "wo": _tile_rows(wo, NCH, TC),
        })

    in_maps = []
    for c in range(8):
        b, g = c // 2, c % 2
        xh_t, xl_t = xprep[b]
        m = {"xh": xh_t, "xl": xl_t, "masks": masks, "ident": ident}
        m.update(wprep[g])
        in_maps.append(m)
    return in_maps


def run(x, Wq, Wk, Wv, Wo, trace=False, **kw):
    if "nc" not in _cached:
        _cached["nc"] = build_nc()
    nc = _cached["nc"]
    in_maps = _prep_inputs(x, Wq, Wk, Wv, Wo)
    res = run_bass_kernel_spmd(nc, in_maps, core_ids=list(range(8)), trace=trace, **kw)
    if USE_COLLECTIVE:
        out = np.stack([res.results[2 * b]["out"] for b in range(B)])
    else:
        out = np.stack([res.results[2 * b]["out"] + res.results[2 * b + 1]["out"]
                        for b in range(B)])
    return out.astype(np.float32), res


def kernel(x, Wq, Wk, Wv, Wo):
    out, _ = run(x, Wq, Wk, Wv, Wo)
    return out
